# revision 14
# baseline (speedup 1.0000x reference)
"""Trainium2 Bass kernel for nn_AttentionBlock_33724083208839 (sparse_attention).

Data-parallel over batch (8 batches -> 8 cores). Per core:
  1. load x (chunked), transpose via PE -> xT f32; project K in f32 (exact,
     feeds selection), Q via fp32r (1 cyc/row), V in f32 -> bf16 vp.
  2. K_reduce via the exact CVaR identity sum_top_l = l*t + sum(relu(x-t)),
     with t from Gaussian quantile + one Newton step on the exact count.
  3. sqk = x @ (Wq @ K_reduce) in f32 on PE; query top-l as a mask via a
     5-pass 128-ary threshold search on a partition-replicated copy of sqk.
  4. attention for all 4096 queries: scores^T on PE in fp32r (1 cyc/row,
     no bf16 conversion copies) -> exp split across ACT (exact, scale=1/8)
     and DVE (Schraudolph bitcast exp: int16(A*s+B) reinterpreted as bf16)
     -> P^T strips bf16 -> reversed AV: lhsT = P^T tile (stationary),
     rhs = [V|1] (65 cols moving) accumulated over 32 k-tiles, so the
     output lands directly in [query-partition, dv] layout; normalize with
     the ones-column denominator, blend with meanV by the mask, DMA out.
"""
import sys

sys.path.insert(0, "/opt/trn_rl_repo")

import math
from statistics import NormalDist

import numpy as np

import concourse.bacc as bacc
import concourse.bass as bass
import concourse.bass_isa as bass_isa
import concourse.mybir as mybir
from concourse.tile import TileContext
from concourse.masks import make_identity
from concourse.bass_utils import run_bass_kernel_spmd

B, L, D = 8, 4096, 64
LQ = int((1.0 - 0.33) * L)  # 2744
PART = 128
NT = L // PART
NS = L // 512
N_CORES = 8

QFRAC = 1.0 - LQ / L
Z = NormalDist().inv_cdf(QFRAC)
PHI = math.exp(-Z * Z / 2.0) / math.sqrt(2.0 * math.pi)

f32 = mybir.dt.float32
f32r = mybir.dt.float32r
bf16 = mybir.dt.bfloat16
u8 = mybir.dt.uint8
i16 = mybir.dt.int16
i32 = mybir.dt.int32
AF = mybir.ActivationFunctionType
OP = mybir.AluOpType

N_PASS = 5
BOUND = 512.0

# Schraudolph exp for bf16 bit patterns: bf16_bits(exp(s/8)) ~= A*s + B.
# A = 128*log2(e)/8; B centers the piecewise-linear sawtooth (mean-unbiased)
# and adds +0.5 to compensate truncation in the float->int16 convert.
SCH_A = 128.0 * math.log2(math.e) / 8.0
SCH_B = 16256.0 + 0.5 - 128.0 * math.log2(1.0407)

GROUPS = [(g, min(3, NT - g)) for g in range(0, NT, 3)]

# exp-engine split: selection runs on GPSIMD, so the DVE can take a fixed
# share of the exp strips (Schraudolph) from slab 0 onward.
DVE_GROUPS = {7, 8, 9, 10}
DVE_FROM_SLAB = 1
DVE_GROUPS_S0 = {9, 10}


def build(debug: bool = False):
    nc = bacc.Bacc("TRN2")
    x = nc.dram_tensor("x", [L, D], f32, kind="ExternalInput")
    wq = nc.dram_tensor("Wq", [D, D], f32, kind="ExternalInput")
    wk = nc.dram_tensor("Wk", [D, D], f32, kind="ExternalInput")
    wv = nc.dram_tensor("Wv", [D, D], f32, kind="ExternalInput")
    out = nc.dram_tensor("out", [L, D], f32, kind="ExternalOutput")
    scr_row = nc.dram_tensor("scr_row", [1, L], f32, kind="Internal")
    dbg = {}
    if debug:
        for name, shape in [
            ("dbg_kr", [D, 1]), ("dbg_thr", [PART, 1]), ("dbg_sqk", [PART, NT]),
            ("dbg_mask", [PART, NT]), ("dbg_tk", [PART, 1]), ("dbg_cnt", [PART, 1]),
        ]:
            dbg[name] = nc.dram_tensor(name, shape, f32, kind="ExternalOutput")

    x_re = x[:].rearrange("(c p) d -> p c d", p=PART)
    out_re = out[:].rearrange("(c p) d -> p c d", p=PART)

    with TileContext(nc) as tc, \
         tc.tile_pool(name="cst", bufs=1) as cst, \
         tc.tile_pool(name="big", bufs=1) as big, \
         tc.tile_pool(name="sc", bufs=2) as sc, \
         tc.tile_pool(name="mn", bufs=2) as mn:

        # ---- warm the exp activation table immediately ----
        warm = cst.tile([1, 8], f32)
        nc.vector.memset(warm[:], 0.0)
        warm2 = cst.tile([1, 8], f32)
        nc.scalar.activation(out=warm2[:], in_=warm[:], func=AF.Exp)

        # ---- constants ----
        ident = cst.tile([PART, PART], f32)
        make_identity(nc, ident[:])
        onesb = cst.tile([PART, 1], bf16)
        nc.vector.memset(onesb[:], 1.0)
        ones1x128 = cst.tile([1, PART], f32)
        nc.vector.memset(ones1x128[:], 1.0)
        pidx1i = cst.tile([PART, 1], i32)
        nc.gpsimd.iota(pidx1i[:], pattern=[[1, 1]], base=1, channel_multiplier=1)
        pidx1 = cst.tile([PART, 1], f32)
        nc.vector.tensor_copy(pidx1[:], pidx1i[:])

        # ---- persistent tensors ----
        x_sb = big.tile([PART, NT, D], f32)
        xT32 = big.tile([D, L], f32)
        qT32 = big.tile([D, L], f32)
        kT32 = big.tile([D, L], f32)
        vp = big.tile([PART, NT, D + 1], bf16)
        pt_a = big.tile([PART, NT, 512], bf16)
        pt_b = big.tile([PART, NT, 512], bf16)
        res = big.tile([PART, NT, D], f32)
        mvf = big.tile([PART, D], f32)
        mask = big.tile([PART, NT], f32)
        inv_u8 = big.tile([PART, NT], u8)
        sqk = big.tile([PART, NT], f32)
        kr = big.tile([D, 1], f32)
        wvec = big.tile([D, 1], f32)
        tmp1m = big.tile([D, L], f32)
        sqk_rep = big.tile([PART, L], f32)
        cmp_rep = big.tile([PART, L], f32)

        # weights
        wq_s = cst.tile([D, D], f32)
        wk_s = cst.tile([D, D], f32)
        wv_s = cst.tile([D, D], f32)
        nc.sync.dma_start(out=wq_s[:], in_=wq[:])
        nc.sync.dma_start(out=wk_s[:], in_=wk[:])
        nc.sync.dma_start(out=wv_s[:], in_=wv[:])

        def kv_r(ap):  # fp32r view for fast PE (1 cyc/row at >=256 cols)
            return ap.bitcast(f32r)

        # =============== phase 1: load / project / slab-0 scores+exp ===============
        with tc.tile_pool(name="ps_xv", bufs=2, space="PSUM") as ps_xv, \
             tc.tile_pool(name="ps_pj", bufs=2, space="PSUM") as ps_pj, \
             tc.tile_pool(name="ps_s0", bufs=1, space="PSUM") as ps_s0:

            def load_tiles(c0, c1):
                nc.sync.dma_start(out=x_sb[:, c0:c1, :], in_=x_re[:, c0:c1, :])
                for c in range(c0, c1):
                    pxt = ps_xv.tile([PART, PART], f32, tag="xv")
                    nc.tensor.transpose(out=pxt[0:D, :], in_=x_sb[:, c, :],
                                        identity=ident[:])
                    nc.vector.tensor_copy(xT32[:, PART * c:PART * (c + 1)], pxt[0:D, :])

            def proj_slab(s):
                sl = slice(512 * s, 512 * (s + 1))
                pk = ps_pj.tile([D, 512], f32, tag="pj")
                nc.tensor.matmul(out=pk[:], lhsT=wk_s[:], rhs=xT32[:, sl],
                                 start=True, stop=True)
                nc.vector.tensor_copy(kT32[:, sl], pk[:])
                pq = ps_pj.tile([D, 512], f32, tag="pj")
                nc.tensor.matmul(out=pq[:], lhsT=kv_r(wq_s[:]), rhs=kv_r(xT32[:, sl]),
                                 start=True, stop=True)
                nc.vector.tensor_copy(qT32[:, sl], pq[:])

            def sg0(gi):
                g0, glen = GROUPS[gi]
                strip = ps_s0.tile([PART, 3, 512], f32, tag="s0")
                for i in range(glen):
                    j = g0 + i
                    nc.tensor.matmul(out=strip[:, i, :],
                                     lhsT=kv_r(kT32[:, PART * j:PART * (j + 1)]),
                                     rhs=kv_r(qT32[:, 0:512]), start=True, stop=True)
                if gi in DVE_GROUPS_S0:
                    nc.vector.tensor_scalar(
                        out=pt_a[:, g0:g0 + glen, :].bitcast(i16),
                        in0=strip[:, 0:glen, :], scalar1=SCH_A, scalar2=SCH_B,
                        op0=OP.mult, op1=OP.add)
                else:
                    nc.scalar.activation(out=pt_a[:, g0:g0 + glen, :],
                                         in_=strip[:, 0:glen, :], func=AF.Exp, scale=0.125)

            def proj_v(c0, c1):
                for c in range(c0, c1):
                    pv = ps_xv.tile([PART, PART], f32, tag="xv")
                    nc.tensor.matmul(out=pv[:, 0:D],
                                     lhsT=xT32[:, PART * c:PART * (c + 1)],
                                     rhs=wv_s[:], start=True, stop=True)
                    nc.vector.tensor_copy(vp[:, c, 0:D], pv[:, 0:D])

            load_tiles(0, 2)
            load_tiles(2, 4)
            proj_slab(0)
            proj_v(0, 4)
            sg0(0)
            load_tiles(4, 8)
            proj_slab(1)
            proj_v(4, 8)
            sg0(1)
            load_tiles(8, 12)
            load_tiles(12, 16)
            proj_slab(2); proj_v(8, 12); sg0(2)
            proj_slab(3); proj_v(12, 16); sg0(3); sg0(4)
            load_tiles(16, 20)
            load_tiles(20, 24)
            proj_slab(4); proj_v(16, 20); sg0(5)
            proj_slab(5); proj_v(20, 24); sg0(6); sg0(7)
            load_tiles(24, 28)
            load_tiles(28, 32)
            proj_slab(6); proj_v(24, 28); sg0(8)
            proj_slab(7); proj_v(28, 32)
            nc.vector.memset(vp[:, :, D:D + 1], 1.0)
            sg0(9)
            sg0(10)

        # ---- K_reduce (DVE only; channel = partition of kT32) ----
        bstats = sc.tile([D, 8, 6], f32, tag="bstats")
        for a in range(8):
            nc.vector.bn_stats(bstats[:, a, :], kT32[:, 512 * a:512 * (a + 1)])
        aggr = sc.tile([D, 2], f32, tag="aggr")
        nc.vector.bn_aggr(aggr[:], bstats[:])
        sig = sc.tile([D, 1], f32, tag="sig")
        nc.vector.memset(sig[:], 1.0)
        for _ in range(4):
            rs = sc.tile([D, 1], f32, tag="rs")
            nc.vector.reciprocal(rs[:], sig[:])
            nc.vector.tensor_tensor(out=rs[:], in0=rs[:], in1=aggr[:, 1:2], op=OP.mult)
            nc.vector.tensor_tensor(out=rs[:], in0=rs[:], in1=sig[:], op=OP.add)
            nc.vector.tensor_scalar_mul(sig[:], rs[:], 0.5)
        tk = sc.tile([D, 1], f32, tag="tk")
        nc.vector.tensor_scalar(out=tk[:], in0=sig[:], scalar1=float(Z),
                                scalar2=None, op0=OP.mult)
        nc.vector.tensor_tensor(out=tk[:], in0=tk[:], in1=aggr[:, 0:1], op=OP.add)
        cnt_c = sc.tile([D, 1], f32, tag="cnt_c")
        nc.gpsimd.tensor_scalar(out=tmp1m[:], in0=kT32[:], scalar1=tk[:, 0:1],
                                scalar2=None, op0=OP.is_gt, op1=OP.add,
                                accum_out=cnt_c[:])
        adj = sc.tile([D, 1], f32, tag="adj")
        nc.vector.tensor_scalar(out=adj[:], in0=cnt_c[:], scalar1=float(-LQ),
                                scalar2=1.0 / (L * PHI), op0=OP.add, op1=OP.mult)
        nc.vector.tensor_tensor(out=adj[:], in0=adj[:], in1=sig[:], op=OP.mult)
        t1 = sc.tile([D, 1], f32, tag="t1")
        nc.vector.tensor_tensor(out=t1[:], in0=tk[:], in1=adj[:], op=OP.add)
        nc.gpsimd.tensor_scalar(out=tmp1m[:], in0=kT32[:], scalar1=t1[:, 0:1],
                                scalar2=0.0, op0=OP.subtract, op1=OP.max)
        s1c = sc.tile([D, 1], f32, tag="s1c")
        nc.gpsimd.tensor_scalar(out=tmp1m[:], in0=tmp1m[:], scalar1=1.0,
                                scalar2=None, op0=OP.mult, op1=OP.add,
                                accum_out=s1c[:])
        nc.vector.tensor_scalar(out=kr[:], in0=s1c[:], scalar1=1.0 / LQ,
                                scalar2=None, op0=OP.mult)
        nc.vector.tensor_tensor(out=kr[:], in0=kr[:], in1=t1[:], op=OP.add)

        # =============== phase 2: attention + selection ===============
        with tc.tile_pool(name="ps_strip", bufs=2, space="PSUM") as ps_strip, \
             tc.tile_pool(name="ps_av", bufs=2, space="PSUM") as ps_av:
            def mis_tile():
                return ps_av.tile([PART, PART], f32, tag="av", name="avm")

            def pt_of(s):
                return pt_a if s % 2 == 0 else pt_b

            # ---- selection PE bits: wvec, sqk, replicate sqk ----
            pwt = mis_tile()
            nc.tensor.transpose(out=pwt[0:D, 0:D], in_=wq_s[:], identity=ident[0:D, 0:D])
            wqT = sc.tile([D, D], f32, tag="wqT")
            nc.vector.tensor_copy(wqT[:], pwt[0:D, 0:D])
            pw = mis_tile()
            nc.tensor.matmul(out=pw[0:D, 0:1], lhsT=wqT[:], rhs=kr[:],
                             start=True, stop=True)
            nc.vector.tensor_copy(wvec[:], pw[0:D, 0:1])
            psq = mis_tile()
            for c in range(NT):
                nc.tensor.matmul(out=psq[:, c:c + 1],
                                 lhsT=xT32[:, PART * c:PART * (c + 1)],
                                 rhs=wvec[:], start=True, stop=True)
            nc.vector.tensor_copy(sqk[:], psq[:, 0:NT])

            psqT = mis_tile()
            nc.tensor.transpose(out=psqT[0:NT, 0:PART], in_=sqk[:], identity=ident[:])
            sqkT = sc.tile([NT, PART], f32, tag="sqkT")
            nc.vector.tensor_copy(sqkT[:], psqT[0:NT, 0:PART])
            nc.sync.dma_start(out=scr_row[:], in_=sqkT[:])
            nc.sync.dma_start(out=sqk_rep[:], in_=scr_row[:].to_broadcast([PART, L]))

            if debug:
                nc.sync.dma_start(out=dbg["dbg_kr"][:], in_=kr[:])
                nc.sync.dma_start(out=dbg["dbg_sqk"][:], in_=sqk[:])
                nc.sync.dma_start(out=dbg["dbg_tk"][0:D, :], in_=t1[:])

            # ---- meanV on PE ----
            pmv = mis_tile()
            for c in range(NT):
                nc.tensor.matmul(out=pmv[0:D + 1, 0:1], lhsT=vp[:, c, :], rhs=onesb[:],
                                 start=(c == 0), stop=(c == NT - 1))
            mv_col = sc.tile([D, 1], f32, tag="mv_col")
            nc.vector.tensor_scalar_mul(mv_col[:], pmv[0:D, 0:1], 1.0 / L)
            pmvT = mis_tile()
            nc.tensor.transpose(out=pmvT[0:1, 0:D], in_=mv_col[:],
                                identity=ident[0:D, 0:D])
            mv_row = sc.tile([1, D], f32, tag="mv_row")
            nc.vector.tensor_copy(mv_row[:], pmvT[0:1, 0:D])
            pmvF = mis_tile()
            nc.tensor.matmul(out=pmvF[:, 0:D], lhsT=ones1x128[:], rhs=mv_row[:],
                             start=True, stop=True)
            nc.vector.tensor_copy(mvf[:], pmvF[:, 0:D])

            # ---- selection: 5-pass 128-ary threshold search, split into
            # stages so the DVE can interleave exp strips between passes ----
            sel_state = {}

            def sel_pass_init():
                lo = mn.tile([PART, 1], f32, tag="lo_a")
                nc.vector.memset(lo[:], -BOUND)
                dlt = mn.tile([PART, 1], f32, tag="dlt_a")
                nc.vector.memset(dlt[:], 2.0 * BOUND / 129.0)
                sel_state["lo"], sel_state["dlt"] = lo, dlt

            def sel_pass(it):
                lo, dlt = sel_state["lo"], sel_state["dlt"]
                tvec = mn.tile([PART, 1], f32, tag=f"tv{it % 2}")
                nc.gpsimd.tensor_tensor(out=tvec[:], in0=pidx1[:], in1=dlt[:], op=OP.mult)
                nc.gpsimd.tensor_tensor(out=tvec[:], in0=tvec[:], in1=lo[:], op=OP.add)
                cntq = mn.tile([PART, 1], f32, tag="cntq")
                nc.gpsimd.tensor_scalar(out=cmp_rep[:], in0=sqk_rep[:],
                                        scalar1=tvec[:, 0:1], scalar2=None,
                                        op0=OP.is_gt, op1=OP.add, accum_out=cntq[:])
                sel = mn.tile([PART, 1], f32, tag="sel")
                nc.gpsimd.tensor_scalar(out=sel[:], in0=cntq[:], scalar1=float(LQ),
                                        scalar2=None, op0=OP.is_ge)
                jsr = mn.tile([PART, 1], f32, tag="jsr")
                nc.gpsimd.partition_all_reduce(jsr[:], sel[:], channels=PART,
                                               reduce_op=bass_isa.ReduceOp.add)
                step = mn.tile([PART, 1], f32, tag="step")
                nc.gpsimd.tensor_tensor(out=step[:], in0=jsr[:], in1=dlt[:], op=OP.mult)
                nlo = mn.tile([PART, 1], f32, tag=f"lo_{'b' if it % 2 == 0 else 'a'}")
                nc.gpsimd.tensor_tensor(out=nlo[:], in0=lo[:], in1=step[:], op=OP.add)
                ndl = mn.tile([PART, 1], f32, tag=f"dlt_{'b' if it % 2 == 0 else 'a'}")
                nc.gpsimd.tensor_scalar_mul(ndl[:], dlt[:], 1.0 / 129.0)
                sel_state["lo"], sel_state["dlt"] = nlo, ndl

            def sel_finish():
                lo = sel_state["lo"]
                nc.gpsimd.tensor_scalar(out=mask[:], in0=sqk[:], scalar1=lo[:, 0:1],
                                        scalar2=None, op0=OP.is_gt)
                # inverted mask as u8 for the meanV blend
                minv = mn.tile([PART, NT], f32, tag="minv")
                nc.gpsimd.tensor_scalar(out=minv[:], in0=mask[:], scalar1=-1.0,
                                        scalar2=1.0, op0=OP.mult, op1=OP.add)
                nc.gpsimd.tensor_copy(inv_u8[:], minv[:])
                if debug:
                    nc.sync.dma_start(out=dbg["dbg_mask"][:], in_=mask[:])
                    nc.sync.dma_start(out=dbg["dbg_thr"][:], in_=lo[:])
                    cntf = mn.tile([PART, 1], f32, tag="cntf")
                    cmpf = mn.tile([PART, NT], f32, tag="cmpf")
                    nc.vector.tensor_scalar(out=cmpf[:], in0=sqk[:], scalar1=lo[:, 0:1],
                                            scalar2=None, op0=OP.is_gt, op1=OP.add,
                                            accum_out=cntf[:])
                    nc.sync.dma_start(out=dbg["dbg_cnt"][:], in_=cntf[:])

            # ---- attention slab machinery ----
            def emit_scores(s, ptc):
                """scores + exp for slab s into ptc strips."""
                for gi, (g0, glen) in enumerate(GROUPS):
                    strip = ps_strip.tile([PART, 3, 512], f32, tag="strip")
                    for i in range(glen):
                        j = g0 + i
                        nc.tensor.matmul(out=strip[:, i, :],
                                         lhsT=kv_r(kT32[:, PART * j:PART * (j + 1)]),
                                         rhs=kv_r(qT32[:, 512 * s:512 * (s + 1)]),
                                         start=True, stop=True)
                    if s >= DVE_FROM_SLAB and gi in DVE_GROUPS:
                        nc.vector.tensor_scalar(
                            out=ptc[:, g0:g0 + glen, :].bitcast(i16),
                            in0=strip[:, 0:glen, :], scalar1=SCH_A, scalar2=SCH_B,
                            op0=OP.mult, op1=OP.add)
                    else:
                        nc.scalar.activation(out=ptc[:, g0:g0 + glen, :],
                                             in_=strip[:, 0:glen, :], func=AF.Exp,
                                             scale=0.125)

            def emit_av(s, ptp):
                """reversed AV for slab s from ptp strips; writes res chunks."""
                for u in range(4):
                    c = 4 * s + u
                    av = mis_tile()
                    for j in range(NT):
                        nc.tensor.matmul(out=av[:, 0:D + 1],
                                         lhsT=ptp[:, j, PART * u:PART * (u + 1)],
                                         rhs=vp[:, j, :],
                                         start=(j == 0), stop=(j == NT - 1))
                    rec = mn.tile([PART, 1], f32, tag="rec")
                    nc.vector.reciprocal_approx_fast(rec[:], av[:, D:D + 1])
                    nc.vector.tensor_scalar(out=res[:, c, :], in0=av[:, 0:D],
                                            scalar1=rec[:, 0:1], scalar2=None,
                                            op0=OP.mult)

            def emit_blend(c):
                nc.vector.copy_predicated(res[:, c, :],
                                          inv_u8[:, c:c + 1].to_broadcast([PART, D]),
                                          mvf[:])
                nc.sync.dma_start(out=out_re[:, c:c + 1, :], in_=res[:, c:c + 1, :])

            # ---- main loop: scores(s) interleaved with AV(s-1) ----
            sel_pass_init()
            sel_pass(0)
            blended = 0
            for s in range(1, NS + 1):
                if s < NS:
                    emit_scores(s, pt_of(s))
                emit_av(s - 1, pt_of(s - 1))
                if s <= N_PASS - 1:
                    sel_pass(s)
                if s == N_PASS:
                    sel_finish()
                if s > N_PASS:
                    # mask is ready; blend everything that has been normalized
                    while blended < 4 * (s - 1):
                        emit_blend(blended)
                        blended += 1
            while blended < NT:
                emit_blend(blended)
                blended += 1

    nc.finalize()
    return nc


_CACHE = {}


def _get_nc(debug=False):
    key = bool(debug)
    if key not in _CACHE:
        _CACHE[key] = build(debug=key)
    return _CACHE[key]


def kernel(x, Wq, Wk, Wv, debug=False):
    nc = _get_nc(debug=debug)
    x = np.asarray(x, dtype=np.float32)
    in_maps = [
        {"x": np.ascontiguousarray(x[i]),
         "Wq": np.asarray(Wq, np.float32), "Wk": np.asarray(Wk, np.float32),
         "Wv": np.asarray(Wv, np.float32)}
        for i in range(B)
    ]
    last_err = None
    for _attempt in range(3):
        try:
            r = run_bass_kernel_spmd(nc, in_maps, core_ids=list(range(N_CORES)))
            out = np.stack([r.results[i]["out"] for i in range(B)]).astype(np.float32)
            break
        except Exception as e:  # transient axon RPC failures
            last_err = e
    else:
        raise last_err
    if debug:
        return out, r.results
    return out


# revision 15
# speedup vs baseline: 1.0791x; 1.0791x over previous
"""Trainium2 Bass kernel for nn_AttentionBlock_33724083208839 (sparse_attention).

Data-parallel over batch (8 batches -> 8 cores). Per core:
  1. load x (chunked), transpose via PE -> xT f32; project K in f32 (exact,
     feeds selection), Q via fp32r (1 cyc/row), V in f32 -> bf16 vp.
  2. K_reduce via the exact CVaR identity sum_top_l = l*t + sum(relu(x-t)),
     with t from Gaussian quantile + one Newton step on the exact count.
  3. sqk = x @ (Wq @ K_reduce) in f32 on PE; query top-l as a mask via a
     5-pass 128-ary threshold search on a partition-replicated copy of sqk.
  4. attention for all 4096 queries: scores^T on PE in fp32r (1 cyc/row,
     no bf16 conversion copies) -> exp split across ACT (exact, scale=1/8)
     and DVE (Schraudolph bitcast exp: int16(A*s+B) reinterpreted as bf16)
     -> P^T strips bf16 -> reversed AV: lhsT = P^T tile (stationary),
     rhs = [V|1] (65 cols moving) accumulated over 32 k-tiles, so the
     output lands directly in [query-partition, dv] layout; normalize with
     the ones-column denominator, blend with meanV by the mask, DMA out.
"""
import sys

sys.path.insert(0, "/opt/trn_rl_repo")

import math
from statistics import NormalDist

import numpy as np

import concourse.bacc as bacc
import concourse.bass as bass
import concourse.bass_isa as bass_isa
import concourse.mybir as mybir
from concourse.tile import TileContext
from concourse.masks import make_identity
from concourse.bass_utils import run_bass_kernel_spmd

B, L, D = 8, 4096, 64
LQ = int((1.0 - 0.33) * L)  # 2744
PART = 128
NT = L // PART
NS = L // 512
N_CORES = 8

QFRAC = 1.0 - LQ / L
Z = NormalDist().inv_cdf(QFRAC)
PHI = math.exp(-Z * Z / 2.0) / math.sqrt(2.0 * math.pi)

f32 = mybir.dt.float32
f32r = mybir.dt.float32r
bf16 = mybir.dt.bfloat16
u8 = mybir.dt.uint8
i16 = mybir.dt.int16
i32 = mybir.dt.int32
AF = mybir.ActivationFunctionType
OP = mybir.AluOpType

N_PASS = 5
BOUND = 512.0

# Schraudolph exp for bf16 bit patterns: bf16_bits(exp(s/8)) ~= A*s + B.
# A = 128*log2(e)/8; B centers the piecewise-linear sawtooth (mean-unbiased)
# and adds +0.5 to compensate truncation in the float->int16 convert.
SCH_A = 128.0 * math.log2(math.e) / 8.0
SCH_B = 16256.0 + 0.5 - 128.0 * math.log2(1.0407)

GROUPS = [(g, min(3, NT - g)) for g in range(0, NT, 3)]

# exp-engine split: selection runs on GPSIMD, so the DVE takes a fixed share
# of the exp strips (Schraudolph), spread through the slab so both engines
# drain the strip ring concurrently.
DVE_GROUPS = {2, 5, 8, 10}
DVE_FROM_SLAB = 1
DVE_GROUPS_S0 = {2, 5, 8, 10}


def build(debug: bool = False):
    nc = bacc.Bacc("TRN2")
    x = nc.dram_tensor("x", [L, D], f32, kind="ExternalInput")
    wq = nc.dram_tensor("Wq", [D, D], f32, kind="ExternalInput")
    wk = nc.dram_tensor("Wk", [D, D], f32, kind="ExternalInput")
    wv = nc.dram_tensor("Wv", [D, D], f32, kind="ExternalInput")
    out = nc.dram_tensor("out", [L, D], f32, kind="ExternalOutput")
    scr_row = nc.dram_tensor("scr_row", [1, L], f32, kind="Internal")
    dbg = {}
    if debug:
        for name, shape in [
            ("dbg_kr", [D, 1]), ("dbg_thr", [PART, 1]), ("dbg_sqk", [PART, NT]),
            ("dbg_mask", [PART, NT]), ("dbg_tk", [PART, 1]), ("dbg_cnt", [PART, 1]),
        ]:
            dbg[name] = nc.dram_tensor(name, shape, f32, kind="ExternalOutput")

    x_re = x[:].rearrange("(c p) d -> p c d", p=PART)
    out_re = out[:].rearrange("(c p) d -> p c d", p=PART)

    with TileContext(nc) as tc, \
         tc.tile_pool(name="cst", bufs=1) as cst, \
         tc.tile_pool(name="big", bufs=1) as big, \
         tc.tile_pool(name="sc", bufs=2) as sc, \
         tc.tile_pool(name="mn", bufs=2) as mn:

        # ---- warm the exp activation table immediately ----
        warm = cst.tile([1, 8], f32)
        nc.vector.memset(warm[:], 0.0)
        warm2 = cst.tile([1, 8], f32)
        nc.scalar.activation(out=warm2[:], in_=warm[:], func=AF.Exp)

        # ---- constants ----
        ident = cst.tile([PART, PART], f32)
        make_identity(nc, ident[:])
        onesb = cst.tile([PART, 1], bf16)
        nc.vector.memset(onesb[:], 1.0)
        ones1x128 = cst.tile([1, PART], f32)
        nc.vector.memset(ones1x128[:], 1.0)
        pidx1i = cst.tile([PART, 1], i32)
        nc.gpsimd.iota(pidx1i[:], pattern=[[1, 1]], base=1, channel_multiplier=1)
        pidx1 = cst.tile([PART, 1], f32)
        nc.vector.tensor_copy(pidx1[:], pidx1i[:])

        # ---- persistent tensors ----
        x_sb = big.tile([PART, NT, D], f32)
        xT32 = big.tile([D, L], f32)
        qT32 = big.tile([D, L], f32)
        kT32 = big.tile([D, L], f32)
        vp = big.tile([PART, NT, D + 1], bf16)
        pt_a = big.tile([PART, NT, 512], bf16)
        pt_b = big.tile([PART, NT, 512], bf16)
        res = big.tile([PART, NT, D], f32)
        mvf = big.tile([PART, D], f32)
        mask = big.tile([PART, NT], f32)
        inv_u8 = big.tile([PART, NT], u8)
        sqk = big.tile([PART, NT], f32)
        kr = big.tile([D, 1], f32)
        wvec = big.tile([D, 1], f32)
        tmp1m = big.tile([D, L], f32)
        sqk_rep = big.tile([PART, L], f32)
        cmp_rep = big.tile([PART, L], f32)

        # weights
        wq_s = cst.tile([D, D], f32)
        wk_s = cst.tile([D, D], f32)
        wv_s = cst.tile([D, D], f32)
        nc.sync.dma_start(out=wq_s[:], in_=wq[:])
        nc.sync.dma_start(out=wk_s[:], in_=wk[:])
        nc.sync.dma_start(out=wv_s[:], in_=wv[:])

        def kv_r(ap):  # fp32r view for fast PE (1 cyc/row at >=256 cols)
            return ap.bitcast(f32r)

        # =============== phase 1: load / project / slab-0 scores+exp ===============
        with tc.tile_pool(name="ps_xv", bufs=2, space="PSUM") as ps_xv, \
             tc.tile_pool(name="ps_pj", bufs=2, space="PSUM") as ps_pj, \
             tc.tile_pool(name="ps_s0", bufs=1, space="PSUM") as ps_s0:

            def load_tiles(c0, c1):
                for c in range(c0, c1):
                    pxt = ps_xv.tile([PART, PART], f32, tag="xv")
                    nc.tensor.transpose(out=pxt[0:D, :], in_=x_sb[:, c, :],
                                        identity=ident[:])
                    nc.vector.tensor_copy(xT32[:, PART * c:PART * (c + 1)], pxt[0:D, :])

            def proj_slab(s):
                sl = slice(512 * s, 512 * (s + 1))
                pk = ps_pj.tile([D, 512], f32, tag="pj")
                nc.tensor.matmul(out=pk[:], lhsT=wk_s[:], rhs=xT32[:, sl],
                                 start=True, stop=True)
                nc.vector.tensor_copy(kT32[:, sl], pk[:])
                pq = ps_pj.tile([D, 512], f32, tag="pj")
                nc.tensor.matmul(out=pq[:], lhsT=kv_r(wq_s[:]), rhs=kv_r(xT32[:, sl]),
                                 start=True, stop=True)
                nc.vector.tensor_copy(qT32[:, sl], pq[:])

            def sg0(gi):
                g0, glen = GROUPS[gi]
                strip = ps_s0.tile([PART, 3, 512], f32, tag="s0")
                for i in range(glen):
                    j = g0 + i
                    nc.tensor.matmul(out=strip[:, i, :],
                                     lhsT=kv_r(kT32[:, PART * j:PART * (j + 1)]),
                                     rhs=kv_r(qT32[:, 0:512]), start=True, stop=True)
                if gi in DVE_GROUPS_S0:  # slab 0
                    nc.vector.tensor_scalar(
                        out=pt_a[:, g0:g0 + glen, :].bitcast(i16),
                        in0=strip[:, 0:glen, :], scalar1=SCH_A, scalar2=SCH_B,
                        op0=OP.mult, op1=OP.add)
                else:
                    nc.scalar.activation(out=pt_a[:, g0:g0 + glen, :],
                                         in_=strip[:, 0:glen, :], func=AF.Exp, scale=0.125)

            def proj_v(c0, c1):
                for c in range(c0, c1):
                    pv = ps_xv.tile([PART, PART], f32, tag="xv")
                    nc.tensor.matmul(out=pv[:, 0:D],
                                     lhsT=xT32[:, PART * c:PART * (c + 1)],
                                     rhs=wv_s[:], start=True, stop=True)
                    nc.vector.tensor_copy(vp[:, c, 0:D], pv[:, 0:D])

            for c0 in range(0, NT, 4):
                nc.sync.dma_start(out=x_sb[:, c0:c0 + 4, :], in_=x_re[:, c0:c0 + 4, :])
            load_tiles(0, 2)
            load_tiles(2, 4)
            proj_slab(0)
            proj_v(0, 4)
            sg0(0)
            load_tiles(4, 8)
            proj_slab(1)
            proj_v(4, 8)
            sg0(1)
            load_tiles(8, 12)
            load_tiles(12, 16)
            proj_slab(2); proj_v(8, 12); sg0(2)
            proj_slab(3); proj_v(12, 16); sg0(3); sg0(4)
            load_tiles(16, 20)
            load_tiles(20, 24)
            proj_slab(4); proj_v(16, 20); sg0(5)
            proj_slab(5); proj_v(20, 24); sg0(6); sg0(7)
            load_tiles(24, 28)
            load_tiles(28, 32)
            proj_slab(6); proj_v(24, 28); sg0(8)
            proj_slab(7); proj_v(28, 32)
            nc.vector.memset(vp[:, :, D:D + 1], 1.0)
            sg0(9)
            sg0(10)

        # ---- K_reduce (DVE only; channel = partition of kT32) ----
        bstats = sc.tile([D, 8, 6], f32, tag="bstats")
        for a in range(8):
            nc.vector.bn_stats(bstats[:, a, :], kT32[:, 512 * a:512 * (a + 1)])
        aggr = sc.tile([D, 2], f32, tag="aggr")
        nc.vector.bn_aggr(aggr[:], bstats[:])
        sig = sc.tile([D, 1], f32, tag="sig")
        nc.vector.memset(sig[:], 1.0)
        for _ in range(4):
            rs = sc.tile([D, 1], f32, tag="rs")
            nc.vector.reciprocal(rs[:], sig[:])
            nc.vector.tensor_tensor(out=rs[:], in0=rs[:], in1=aggr[:, 1:2], op=OP.mult)
            nc.vector.tensor_tensor(out=rs[:], in0=rs[:], in1=sig[:], op=OP.add)
            nc.vector.tensor_scalar_mul(sig[:], rs[:], 0.5)
        tk = sc.tile([D, 1], f32, tag="tk")
        nc.vector.tensor_scalar(out=tk[:], in0=sig[:], scalar1=float(Z),
                                scalar2=None, op0=OP.mult)
        nc.vector.tensor_tensor(out=tk[:], in0=tk[:], in1=aggr[:, 0:1], op=OP.add)
        cnt_c = sc.tile([D, 1], f32, tag="cnt_c")
        nc.gpsimd.tensor_scalar(out=tmp1m[:], in0=kT32[:], scalar1=tk[:, 0:1],
                                scalar2=None, op0=OP.is_gt, op1=OP.add,
                                accum_out=cnt_c[:])
        adj = sc.tile([D, 1], f32, tag="adj")
        nc.vector.tensor_scalar(out=adj[:], in0=cnt_c[:], scalar1=float(-LQ),
                                scalar2=1.0 / (L * PHI), op0=OP.add, op1=OP.mult)
        nc.vector.tensor_tensor(out=adj[:], in0=adj[:], in1=sig[:], op=OP.mult)
        t1 = sc.tile([D, 1], f32, tag="t1")
        nc.vector.tensor_tensor(out=t1[:], in0=tk[:], in1=adj[:], op=OP.add)
        nc.gpsimd.tensor_scalar(out=tmp1m[:], in0=kT32[:], scalar1=t1[:, 0:1],
                                scalar2=0.0, op0=OP.subtract, op1=OP.max)
        s1c = sc.tile([D, 1], f32, tag="s1c")
        nc.gpsimd.tensor_scalar(out=tmp1m[:], in0=tmp1m[:], scalar1=1.0,
                                scalar2=None, op0=OP.mult, op1=OP.add,
                                accum_out=s1c[:])
        nc.vector.tensor_scalar(out=kr[:], in0=s1c[:], scalar1=1.0 / LQ,
                                scalar2=None, op0=OP.mult)
        nc.vector.tensor_tensor(out=kr[:], in0=kr[:], in1=t1[:], op=OP.add)

        # =============== phase 2: attention + selection ===============
        with tc.tile_pool(name="ps_strip", bufs=2, space="PSUM") as ps_strip, \
             tc.tile_pool(name="ps_av", bufs=2, space="PSUM") as ps_av:
            def mis_tile():
                return ps_av.tile([PART, PART], f32, tag="av", name="avm")

            def pt_of(s):
                return pt_a if s % 2 == 0 else pt_b

            # ---- selection PE bits: wvec, sqk, replicate sqk ----
            pwt = mis_tile()
            nc.tensor.transpose(out=pwt[0:D, 0:D], in_=wq_s[:], identity=ident[0:D, 0:D])
            wqT = sc.tile([D, D], f32, tag="wqT")
            nc.vector.tensor_copy(wqT[:], pwt[0:D, 0:D])
            pw = mis_tile()
            nc.tensor.matmul(out=pw[0:D, 0:1], lhsT=wqT[:], rhs=kr[:],
                             start=True, stop=True)
            nc.vector.tensor_copy(wvec[:], pw[0:D, 0:1])
            psq = mis_tile()
            for c in range(NT):
                nc.tensor.matmul(out=psq[:, c:c + 1],
                                 lhsT=xT32[:, PART * c:PART * (c + 1)],
                                 rhs=wvec[:], start=True, stop=True)
            nc.vector.tensor_copy(sqk[:], psq[:, 0:NT])

            psqT = mis_tile()
            nc.tensor.transpose(out=psqT[0:NT, 0:PART], in_=sqk[:], identity=ident[:])
            sqkT = sc.tile([NT, PART], f32, tag="sqkT")
            nc.vector.tensor_copy(sqkT[:], psqT[0:NT, 0:PART])
            nc.sync.dma_start(out=scr_row[:], in_=sqkT[:])
            nc.sync.dma_start(out=sqk_rep[:], in_=scr_row[:].to_broadcast([PART, L]))

            if debug:
                nc.sync.dma_start(out=dbg["dbg_kr"][:], in_=kr[:])
                nc.sync.dma_start(out=dbg["dbg_sqk"][:], in_=sqk[:])
                nc.sync.dma_start(out=dbg["dbg_tk"][0:D, :], in_=t1[:])

            # ---- meanV on PE ----
            pmv = mis_tile()
            for c in range(NT):
                nc.tensor.matmul(out=pmv[0:D + 1, 0:1], lhsT=vp[:, c, :], rhs=onesb[:],
                                 start=(c == 0), stop=(c == NT - 1))
            mv_col = sc.tile([D, 1], f32, tag="mv_col")
            nc.vector.tensor_scalar_mul(mv_col[:], pmv[0:D, 0:1], 1.0 / L)
            pmvT = mis_tile()
            nc.tensor.transpose(out=pmvT[0:1, 0:D], in_=mv_col[:],
                                identity=ident[0:D, 0:D])
            mv_row = sc.tile([1, D], f32, tag="mv_row")
            nc.vector.tensor_copy(mv_row[:], pmvT[0:1, 0:D])
            pmvF = mis_tile()
            nc.tensor.matmul(out=pmvF[:, 0:D], lhsT=ones1x128[:], rhs=mv_row[:],
                             start=True, stop=True)
            nc.vector.tensor_copy(mvf[:], pmvF[:, 0:D])

            # ---- selection: 5-pass 128-ary threshold search, split into
            # stages so the DVE can interleave exp strips between passes ----
            sel_state = {}

            def sel_pass_init():
                lo = mn.tile([PART, 1], f32, tag="lo_a")
                nc.vector.memset(lo[:], -BOUND)
                dlt = mn.tile([PART, 1], f32, tag="dlt_a")
                nc.vector.memset(dlt[:], 2.0 * BOUND / 129.0)
                sel_state["lo"], sel_state["dlt"] = lo, dlt

            def sel_pass(it):
                lo, dlt = sel_state["lo"], sel_state["dlt"]
                tvec = mn.tile([PART, 1], f32, tag=f"tv{it % 2}")
                nc.gpsimd.tensor_tensor(out=tvec[:], in0=pidx1[:], in1=dlt[:], op=OP.mult)
                nc.gpsimd.tensor_tensor(out=tvec[:], in0=tvec[:], in1=lo[:], op=OP.add)
                cntq = mn.tile([PART, 1], f32, tag="cntq")
                nc.gpsimd.tensor_scalar(out=cmp_rep[:], in0=sqk_rep[:],
                                        scalar1=tvec[:, 0:1], scalar2=None,
                                        op0=OP.is_gt, op1=OP.add, accum_out=cntq[:])
                sel = mn.tile([PART, 1], f32, tag="sel")
                nc.gpsimd.tensor_scalar(out=sel[:], in0=cntq[:], scalar1=float(LQ),
                                        scalar2=None, op0=OP.is_ge)
                jsr = mn.tile([PART, 1], f32, tag="jsr")
                nc.gpsimd.partition_all_reduce(jsr[:], sel[:], channels=PART,
                                               reduce_op=bass_isa.ReduceOp.add)
                step = mn.tile([PART, 1], f32, tag="step")
                nc.gpsimd.tensor_tensor(out=step[:], in0=jsr[:], in1=dlt[:], op=OP.mult)
                nlo = mn.tile([PART, 1], f32, tag=f"lo_{'b' if it % 2 == 0 else 'a'}")
                nc.gpsimd.tensor_tensor(out=nlo[:], in0=lo[:], in1=step[:], op=OP.add)
                ndl = mn.tile([PART, 1], f32, tag=f"dlt_{'b' if it % 2 == 0 else 'a'}")
                nc.gpsimd.tensor_scalar_mul(ndl[:], dlt[:], 1.0 / 129.0)
                sel_state["lo"], sel_state["dlt"] = nlo, ndl

            def sel_finish():
                lo = sel_state["lo"]
                nc.gpsimd.tensor_scalar(out=mask[:], in0=sqk[:], scalar1=lo[:, 0:1],
                                        scalar2=None, op0=OP.is_gt)
                # inverted mask as u8 for the meanV blend
                minv = mn.tile([PART, NT], f32, tag="minv")
                nc.gpsimd.tensor_scalar(out=minv[:], in0=mask[:], scalar1=-1.0,
                                        scalar2=1.0, op0=OP.mult, op1=OP.add)
                nc.gpsimd.tensor_copy(inv_u8[:], minv[:])
                if debug:
                    nc.sync.dma_start(out=dbg["dbg_mask"][:], in_=mask[:])
                    nc.sync.dma_start(out=dbg["dbg_thr"][:], in_=lo[:])
                    cntf = mn.tile([PART, 1], f32, tag="cntf")
                    cmpf = mn.tile([PART, NT], f32, tag="cmpf")
                    nc.vector.tensor_scalar(out=cmpf[:], in0=sqk[:], scalar1=lo[:, 0:1],
                                            scalar2=None, op0=OP.is_gt, op1=OP.add,
                                            accum_out=cntf[:])
                    nc.sync.dma_start(out=dbg["dbg_cnt"][:], in_=cntf[:])

            # ---- attention slab machinery ----
            def score_group(s, ptc, gi):
                g0, glen = GROUPS[gi]
                strip = ps_strip.tile([PART, 3, 512], f32, tag="strip")
                for i in range(glen):
                    j = g0 + i
                    nc.tensor.matmul(out=strip[:, i, :],
                                     lhsT=kv_r(kT32[:, PART * j:PART * (j + 1)]),
                                     rhs=kv_r(qT32[:, 512 * s:512 * (s + 1)]),
                                     start=True, stop=True)
                if s >= DVE_FROM_SLAB and gi in DVE_GROUPS:
                    nc.vector.tensor_scalar(
                        out=ptc[:, g0:g0 + glen, :].bitcast(i16),
                        in0=strip[:, 0:glen, :], scalar1=SCH_A, scalar2=SCH_B,
                        op0=OP.mult, op1=OP.add)
                else:
                    nc.scalar.activation(out=ptc[:, g0:g0 + glen, :],
                                         in_=strip[:, 0:glen, :], func=AF.Exp,
                                         scale=0.125)

            def av_subtile(s, ptp, u):
                c = 4 * s + u
                av = mis_tile()
                for j in range(NT):
                    nc.tensor.matmul(out=av[:, 0:D + 1],
                                     lhsT=ptp[:, j, PART * u:PART * (u + 1)],
                                     rhs=vp[:, j, :],
                                     start=(j == 0), stop=(j == NT - 1))
                rec = mn.tile([PART, 1], f32, tag="rec")
                nc.vector.reciprocal_approx_fast(rec[:], av[:, D:D + 1])
                nc.vector.tensor_scalar(out=res[:, c, :], in0=av[:, 0:D],
                                        scalar1=rec[:, 0:1], scalar2=None,
                                        op0=OP.mult)

            def emit_slab(s):
                """scores+exp of slab s (if any) interleaved with AV of s-1."""
                ptc, ptp = pt_of(s), pt_of(s - 1)
                for gi in range(len(GROUPS)):
                    if s < NS:
                        score_group(s, ptc, gi)
                    if gi in (1, 3, 5, 7):
                        av_subtile(s - 1, ptp, (gi - 1) // 2)

            def emit_blend(c):
                # res = mask * (res - mvf) + mvf, on the (idle) GPSIMD
                maskb = mask[:, c:c + 1].to_broadcast([PART, D])
                tb = mn.tile([PART, D], f32, tag="tb")
                nc.gpsimd.tensor_tensor(out=tb[:], in0=res[:, c, :], in1=mvf[:],
                                        op=OP.subtract)
                nc.gpsimd.tensor_tensor(out=tb[:], in0=tb[:], in1=maskb, op=OP.mult)
                nc.gpsimd.tensor_tensor(out=res[:, c, :], in0=tb[:], in1=mvf[:],
                                        op=OP.add)
                nc.sync.dma_start(out=out_re[:, c:c + 1, :], in_=res[:, c:c + 1, :])

            # ---- main loop: scores(s) interleaved with AV(s-1) ----
            sel_pass_init()
            sel_pass(0)
            blended = 0
            for s in range(1, NS + 1):
                emit_slab(s)
                if s <= N_PASS - 1:
                    sel_pass(s)
                if s == N_PASS:
                    sel_finish()
                if s > N_PASS:
                    # mask is ready; blend everything that has been normalized
                    while blended < 4 * (s - 1):
                        emit_blend(blended)
                        blended += 1
            while blended < NT:
                emit_blend(blended)
                blended += 1

    nc.finalize()
    return nc


_CACHE = {}


def _get_nc(debug=False):
    key = bool(debug)
    if key not in _CACHE:
        _CACHE[key] = build(debug=key)
    return _CACHE[key]


def kernel(x, Wq, Wk, Wv, debug=False):
    nc = _get_nc(debug=debug)
    x = np.asarray(x, dtype=np.float32)
    in_maps = [
        {"x": np.ascontiguousarray(x[i]),
         "Wq": np.asarray(Wq, np.float32), "Wk": np.asarray(Wk, np.float32),
         "Wv": np.asarray(Wv, np.float32)}
        for i in range(B)
    ]
    last_err = None
    for _attempt in range(3):
        try:
            r = run_bass_kernel_spmd(nc, in_maps, core_ids=list(range(N_CORES)))
            out = np.stack([r.results[i]["out"] for i in range(B)]).astype(np.float32)
            break
        except Exception as e:  # transient axon RPC failures
            last_err = e
    else:
        raise last_err
    if debug:
        return out, r.results
    return out
